# revision 7
# baseline (speedup 1.0000x reference)
"""TRN2 Bass kernel for nn_DAGLayer (gnn_message_passing).

DAG of 1x1 convs over [B=64, C=64, H=32, W=32]:
  preproc: s0 = W_pre[0] @ x0, s1 = W_pre[1] @ x1   (channel matmul)
  node i (i=0..3): s_{2+i} = sum_j conv1x1(relu(s_j), W_edge[...]) over all
  prior states j; output = concat(s2..s5) on channels -> [B, 256, H, W].

Strategy: data-parallel over batch across 8 NeuronCores (8 batches/core).
Every 1x1 conv is a channel-dim matmul over N = H*W spatial columns.
Matmul operands are fp16 (fp32 PSUM accumulation); weights are packed
host-side into one [128, 640] lhsT block (see _pack_weights).

v2 layout: one batch = one 1024-column macro tile. PSUM tiles are
[128, 1024] fp32 (two adjacent banks); the two 512-col halves are written
by separate matmuls, and PSUM->SBUF relu/cast ops drain both banks with a
single instruction (halves the fixed per-op cost, which dominates the
Scalar/Vector budget - the true bottleneck since GpSimd has no PSUM port).
Per batch, five 1x-rate PSUM drains (r01, r2, outA, r4, outB) alternate
between Scalar and Vector; relu(s3) is derived from the fp16 outA copy on
the DVE 4x fast path instead of a sixth PSUM read. Batches are processed
in pairs sharing each weight block so same-weight matmuls issue
back-to-back. A short burst of dummy matmuls warms the PE HAM clock-gate
while the first input DMA (HWDGE, issued ahead of the SWDGE bulk loads)
is in flight.
"""
import sys

sys.path.insert(0, '/opt/trn_rl_repo')

import numpy as np

N_CORES = 8
B, C, H, W_SP = 64, 64, 32, 32
BP = B // N_CORES          # batches per core
HW = H * W_SP              # 1024 spatial columns per batch
NCOL = 512                 # matmul free-dim tile (one fp32 PSUM bank)
N_WARM = 6                 # dummy matmuls to open the PE HAM clock gate

# Set by test harness to capture an NTFF trace; harmless default.
TRACE = False
LAST_RESULTS = None

_cache = {}


def _pack_weights(W_pre: np.ndarray, W_edge: np.ndarray) -> np.ndarray:
    """Pack all conv weights into one [128, 640] fp16 lhsT block.

    Layout (cols):
      0:128   WP  block-diag preproc: out [s0; s1] from rhs [x0; x1]
      128:256 A1  out [s2 | s3p] from rhs R01 = [r0; r1]
      256:384 B1  out [s4p | s5p] from rhs R01
      384:512 B2  out [s4p | s5p] from rhs R23 = [r2; r3]
      512:576 A2  (rows 0:64) edge r2->s3, written at PSUM partitions 64:128
      576:640 B3  (rows 0:64) edge r4->s5, written at PSUM partitions 64:128
    lhsT[k, m] = W[m, k] (pre-transposed for the PE's stationary operand).
    """
    Wt = np.zeros((128, 640), np.float32)
    T = lambda w: np.ascontiguousarray(w.T)
    Wt[0:64, 0:64] = T(W_pre[0])
    Wt[64:128, 64:128] = T(W_pre[1])
    # A1: cols 0:64 -> s2 (edges 0(r0), 1(r1)); cols 64:128 -> s3p (2, 3)
    Wt[0:64, 128:192] = T(W_edge[0])
    Wt[64:128, 128:192] = T(W_edge[1])
    Wt[0:64, 192:256] = T(W_edge[2])
    Wt[64:128, 192:256] = T(W_edge[3])
    # B1: cols 0:64 -> s4p (5(r0), 6(r1)); cols 64:128 -> s5p (9, 10)
    Wt[0:64, 256:320] = T(W_edge[5])
    Wt[64:128, 256:320] = T(W_edge[6])
    Wt[0:64, 320:384] = T(W_edge[9])
    Wt[64:128, 320:384] = T(W_edge[10])
    # B2 (rhs [r2; r3]): cols 0:64 -> s4p (7(r2), 8(r3)); cols 64:128 -> s5p (11, 12)
    Wt[0:64, 384:448] = T(W_edge[7])
    Wt[64:128, 384:448] = T(W_edge[8])
    Wt[0:64, 448:512] = T(W_edge[11])
    Wt[64:128, 448:512] = T(W_edge[12])
    # second-tier edges (K=64, weights at rows 0:64)
    Wt[0:64, 512:576] = T(W_edge[4])
    Wt[0:64, 576:640] = T(W_edge[13])
    return Wt.astype(np.float16)


def _build_program():
    import concourse.tile as tile
    from concourse import bacc, mybir

    F16, F32 = mybir.dt.float16, mybir.dt.float32
    Relu = mybir.ActivationFunctionType.Relu

    nc = bacc.Bacc()
    X = nc.dram_tensor("X", [BP, 128, HW], F16, kind="ExternalInput")
    Wt = nc.dram_tensor("Wt", [128, 640], F16, kind="ExternalInput")
    O = nc.dram_tensor("O", [BP, 256, HW], F16, kind="ExternalOutput")

    with tile.TileContext(nc) as tc:
        with tc.tile_pool(name="wpool", bufs=1) as wpool, \
             tc.tile_pool(name="dpool", bufs=1) as dpool, \
             tc.tile_pool(name="xpool", bufs=3) as xpool, \
             tc.tile_pool(name="rpool", bufs=2) as rpool, \
             tc.tile_pool(name="r23pool", bufs=2) as r23pool, \
             tc.tile_pool(name="r4pool", bufs=2) as r4pool, \
             tc.tile_pool(name="oapool", bufs=3) as oapool, \
             tc.tile_pool(name="obpool", bufs=3) as obpool, \
             tc.tile_pool(name="apool", bufs=2, space="PSUM") as apool, \
             tc.tile_pool(name="bpool", bufs=2, space="PSUM") as bpool:
            w = wpool.tile([128, 640], F16, tag="w")
            nc.sync.dma_start(w[:], Wt[:])

            # PE warm-up: the HAM clock gate holds the PE at 1.2 GHz until
            # ~3.4us of sustained busy.  Burn that window on dummy matmuls
            # while the first input DMA is in flight so the real matmuls
            # start at 2.4 GHz.
            dum = dpool.tile([128, NCOL], F16, tag="dum")
            nc.gpsimd.memset(dum[:], 0)
            # shares the pB ring so no extra PSUM bank is committed
            pwarm = bpool.tile([128, HW], F32, tag="pB", name="pwarm")
            for _ in range(N_WARM):
                nc.tensor.matmul(pwarm[:, 0:NCOL], dum[:, 0:128], dum[:],
                                 start=True, stop=True)

            halves = (slice(0, NCOL), slice(NCOL, HW))

            def relu(eng, dst, src):
                if eng == "s":
                    nc.scalar.activation(dst, src, Relu)
                else:
                    nc.vector.tensor_relu(dst, src)

            def cast(eng, dst, src):
                if eng == "s":
                    nc.scalar.copy(dst, src)
                else:
                    nc.vector.tensor_copy(dst, src)

            for p in range(BP // 2):
                bb = (2 * p, 2 * p + 1)
                # engine parity per batch: batch b0 drains r01/r2/r4 on
                # Scalar (outA/outB on Vector), batch b1 the reverse; the
                # r4 of b1 alternates per pair to even out the 5-op split.
                epar = {bb[0]: ("s", "v"), bb[1]: ("v", "s")}
                xs, pP, pA, pB = {}, {}, {}, {}
                r01, r23, r4, oA, oB = {}, {}, {}, {}, {}

                for b in bb:
                    xs[b] = xpool.tile([128, HW], F16, tag="x", name="x")
                    if b < 2:
                        nc.sync.dma_start(xs[b][:], X[b])
                    else:
                        nc.gpsimd.dma_start(xs[b][:], X[b])
                # stage P: [s0; s1] (block-diag preproc)
                for b in bb:
                    # pP shares the "pa" ring with pA: the slot is recycled
                    # as soon as r01 drains it, which is exactly when A1
                    # (the next writer) becomes runnable anyway.
                    pP[b] = apool.tile([128, HW], F32, tag="pa", name="pP")
                    for hs in halves:
                        nc.tensor.matmul(pP[b][:, hs], w[:, 0:128],
                                         xs[b][:, hs], start=True, stop=True)
                for b in bb:
                    r01[b] = rpool.tile([128, HW], F16, tag="r01", name="r01")
                    relu(epar[b][0], r01[b][:], pP[b][:])
                # stage A1: pA = [s2; s3p] from [r0; r1]
                for b in bb:
                    pA[b] = apool.tile([128, HW], F32, tag="pa", name="pA")
                    for hs in halves:
                        nc.tensor.matmul(pA[b][:, hs], w[:, 128:256],
                                         r01[b][:, hs], start=True, stop=False)
                for b in bb:
                    r23[b] = r23pool.tile([128, HW], F16, tag="r23", name="r23")
                    relu(epar[b][0], r23[b][0:64, :], pA[b][0:64, :])   # r2
                # stage B1: pB = [s4p; s5p] from [r0; r1] (keeps the PE busy
                # while the r2 drains run)
                for b in bb:
                    pB[b] = bpool.tile([128, HW], F32, tag="pB", name="pB")
                    for hs in halves:
                        nc.tensor.matmul(pB[b][:, hs], w[:, 256:384],
                                         r01[b][:, hs], start=True, stop=False)
                # stage A2: s3 += E4 @ r2 into PSUM partitions 64:128
                for b in bb:
                    for hs in halves:
                        nc.tensor.matmul(pA[b][64:128, hs], w[0:64, 512:576],
                                         r23[b][0:64, hs], start=False,
                                         stop=True, tile_position=(0, 64))
                # outA = fp16([s2; s3]); r3 from the fp16 copy on the DVE 4x
                # path (fp16 SBUF->SBUF) instead of a 6th 1x PSUM read.
                for b in bb:
                    oA[b] = oapool.tile([128, HW], F16, tag="outA", name="outA")
                    cast(epar[b][1], oA[b][:], pA[b][:])
                for b in bb:
                    nc.vector.tensor_relu(r23[b][64:128, :], oA[b][64:128, :])
                # stage B2: pB += [s4p; s5p] from [r2; r3]
                for b in bb:
                    for hs in halves:
                        nc.tensor.matmul(pB[b][:, hs], w[:, 384:512],
                                         r23[b][:, hs], start=False, stop=False)
                for b in bb:
                    r4[b] = r4pool.tile([64, HW], F16, tag="r4", name="r4")
                    e = epar[b][0] if b == bb[0] or p % 2 == 0 else "s"
                    relu(e, r4[b][:], pB[b][0:64, :])
                # stage B3: s5 += E13 @ r4 into PSUM partitions 64:128
                for b in bb:
                    for hs in halves:
                        nc.tensor.matmul(pB[b][64:128, hs], w[0:64, 576:640],
                                         r4[b][:, hs], start=False,
                                         stop=True, tile_position=(0, 64))
                for b in bb:
                    oB[b] = obpool.tile([128, HW], F16, tag="outB", name="outB")
                    cast(epar[b][1], oB[b][:], pB[b][:])
                # output channel order: s2 | s3 | s4 | s5
                for b in bb:
                    nc.sync.dma_start(O[b, 0:128, :], oA[b][:])
                    nc.sync.dma_start(O[b, 128:256, :], oB[b][:])
    nc.compile()
    return nc


def _get_program():
    if "nc" not in _cache:
        _cache["nc"] = _build_program()
    return _cache["nc"]


def kernel(x0, x1, W_pre, W_edge):
    global LAST_RESULTS
    from concourse.bass_utils import run_bass_kernel_spmd

    nc = _get_program()
    Xp = np.concatenate(
        [x0.reshape(B, C, HW), x1.reshape(B, C, HW)], axis=1)   # [B, 128, HW]
    Xp = Xp.astype(np.float16)
    Wt = _pack_weights(np.asarray(W_pre, np.float32), np.asarray(W_edge, np.float32))
    in_maps = [
        {"X": np.ascontiguousarray(Xp[i * BP:(i + 1) * BP]), "Wt": Wt}
        for i in range(N_CORES)
    ]
    res = run_bass_kernel_spmd(nc, in_maps, core_ids=list(range(N_CORES)),
                               trace=TRACE)
    LAST_RESULTS = res
    out = np.concatenate([r["O"] for r in res.results], axis=0).astype(np.float32)
    return np.ascontiguousarray(out.reshape(B, 4 * C, H, W_SP))
